# revision 1
# baseline (speedup 1.0000x reference)
"""LightGCN 3-layer SpMM on 8 TRN2 NeuronCores.

Row-sharded edge-parallel SpMM: core c owns output rows [c*12500, (c+1)*12500).
Per layer (one SPMD launch): each core SWDGE-gathers x[col] for its edges
(col-chunked to fit int16 indices), scales by edge value on the vector engine,
and SWDGE-scatter-adds into its DRAM row slice. Rows are assigned round-robin
to tiles so no row repeats within one scatter instruction (the HW CCE add is
not atomic for duplicate indices in flight). Rare overflow edges (row degree
beyond tile count) are computed on the host and added exactly.
"""
import sys

sys.path.insert(0, "/opt/trn_rl_repo")
import numpy as np

N_NODES = 100000
DIM = 64
NCORES = 8
NLAYERS = 3
RPC = N_NODES // NCORES          # 12500 rows per core
NCHUNK = 4
CH = N_NODES // NCHUNK           # 25000 col rows per gather chunk (int16-safe)
T = 8192                         # edges per tile (= per gather/scatter instr)
S = T // 128
TPCH = 13                        # tiles per chunk (13*8192 = 106496 >= ~100K+pad)
NT = NCHUNK * TPCH               # 52 tiles per core per layer
SPARE = T                        # spare rows for padding-edge scatter targets
YEXT = RPC + SPARE

_prog_cache = {}


def _build_program():
    if "nc" in _prog_cache:
        return _prog_cache["nc"]
    from concourse import bass, bacc, tile, library_config, mybir

    f32 = mybir.dt.float32
    i16 = mybir.dt.int16
    nc = bacc.Bacc(None, target_bir_lowering=False, debug=False)
    x = nc.dram_tensor("x", [N_NODES, DIM], f32, kind="ExternalInput")
    cidx = nc.dram_tensor("cidx", [NT, 128, T // 16], i16, kind="ExternalInput")
    ridx = nc.dram_tensor("ridx", [NT, 128, T // 16], i16, kind="ExternalInput")
    vals = nc.dram_tensor("vals", [NT, 128, S, 1], f32, kind="ExternalInput")
    y = nc.dram_tensor("y", [YEXT, DIM], f32, kind="ExternalOutput")

    with tile.TileContext(nc) as tc:
        nc.gpsimd.load_library(library_config.mlp)
        with (
            tc.tile_pool(name="ip", bufs=8) as ip,
            tc.tile_pool(name="gp", bufs=6) as gp,
        ):
            t = 0
            for c in range(NCHUNK):
                xc = x[c * CH:(c + 1) * CH, :]
                for _ in range(TPCH):
                    ci = ip.tile([128, T // 16], i16)
                    ri = ip.tile([128, T // 16], i16)
                    vv = ip.tile([128, S, 1], f32)
                    nc.sync.dma_start(ci[:], cidx[t])
                    nc.sync.dma_start(ri[:], ridx[t])
                    nc.sync.dma_start(vv[:], vals[t])
                    g = gp.tile([128, S, DIM], f32)
                    # SWDGE ring holds <2048 descriptors per instruction:
                    # split each 8192-token tile into 1024-token sub-ops
                    SUB = 1024
                    NS = T // SUB          # 8
                    SS = SUB // 128        # 8 slots per sub-op
                    for i in range(NS):
                        nc.gpsimd.dma_gather(
                            g[:, i * SS:(i + 1) * SS, :], xc,
                            ci[:, i * (SUB // 16):(i + 1) * (SUB // 16)],
                            SUB, SUB, DIM,
                        )
                    ga, va = bass.broadcast_tensor_aps(g[:], vv[:])
                    nc.vector.tensor_tensor(ga, ga, va, mybir.AluOpType.mult)
                    for i in range(NS):
                        nc.gpsimd.dma_scatter_add(
                            y[:], g[:, i * SS:(i + 1) * SS, :],
                            ri[:, i * (SUB // 16):(i + 1) * (SUB // 16)],
                            SUB, SUB, DIM,
                        )
                    t += 1
    nc.compile()
    _prog_cache["nc"] = nc
    return nc


def _wrap16(a):
    # [NT, T] -> [NT, 128, T//16]: token j of tile at [j%16, j//16], x8 replicas
    nt = a.shape[0]
    w = a.reshape(nt, T // 16, 16).transpose(0, 2, 1)
    return np.ascontiguousarray(np.tile(w, (1, 8, 1)))


def _prep_core(rows, cols, vvals):
    """rows: local [0,RPC); returns (cidx, ridx, vals arrays, fixup edges)."""
    chunk = cols // CH
    order = np.lexsort((rows, chunk))
    rows, cols, vvals, chunk = rows[order], cols[order], vvals[order], chunk[order]
    # occurrence rank k within each (chunk, row) group
    key = chunk.astype(np.int64) * RPC + rows
    ne = len(key)
    starts = np.flatnonzero(np.r_[True, key[1:] != key[:-1]])
    group_id = np.cumsum(np.r_[True, key[1:] != key[:-1]]) - 1
    k = np.arange(ne) - starts[group_id]
    fix = k >= TPCH
    tile_id = chunk * TPCH + (k + rows) % TPCH
    # drop fixup edges, count per-tile occupancy
    good = ~fix
    tid = tile_id[good]
    # position within tile
    order2 = np.argsort(tid, kind="stable")
    tid_s = tid[order2]
    tstarts = np.searchsorted(tid_s, np.arange(NT))
    tcounts = np.searchsorted(tid_s, np.arange(NT), side="right") - tstarts
    # per-tile overflow beyond T also goes to fixup
    pos_in_tile = np.arange(len(tid_s)) - tstarts[tid_s]
    ovf = pos_in_tile >= T
    # build dense [NT, T] arrays
    cidx_a = np.zeros((NT, T), np.int16)
    ridx_a = (RPC + np.arange(T, dtype=np.int32))[None, :] * np.ones((NT, 1), np.int32)
    vals_a = np.zeros((NT, T), np.float32)
    gi = np.flatnonzero(good)[order2][~ovf]        # original (sorted) edge idx
    tt = tid_s[~ovf]
    pp = pos_in_tile[~ovf]
    cidx_a[tt, pp] = (cols[gi] - chunk[gi] * CH).astype(np.int16)
    ridx_a[tt, pp] = rows[gi]
    vals_a[tt, pp] = vvals[gi]
    ridx_a = ridx_a.astype(np.int16)
    # fixup edges: occurrence >= TPCH or tile overflow
    fixsel = np.zeros(ne, bool)
    fixsel[fix] = True
    if ovf.any():
        fixsel[np.flatnonzero(good)[order2][ovf]] = True
    fx = (rows[fixsel], cols[fixsel], vvals[fixsel])
    vals_w = vals_a.reshape(NT, S, 128).transpose(0, 2, 1)[..., None]
    return (
        _wrap16(cidx_a),
        _wrap16(ridx_a),
        np.ascontiguousarray(vals_w),
        fx,
    )


def _prep(adj_row, adj_col, adj_vals):
    per_core = []
    fix_r, fix_c, fix_v = [], [], []
    core = adj_row // RPC
    for c in range(NCORES):
        sel = core == c
        ci, ri, vv, (fr, fc, fv) = _prep_core(
            (adj_row[sel] - c * RPC).astype(np.int32),
            adj_col[sel].astype(np.int32),
            adj_vals[sel].astype(np.float32),
        )
        per_core.append({"cidx": ci, "ridx": ri, "vals": vv})
        fix_r.append(fr + c * RPC)
        fix_c.append(fc)
        fix_v.append(fv)
    return per_core, np.concatenate(fix_r), np.concatenate(fix_c), np.concatenate(fix_v)


def kernel(user_emb, item_emb, adj_vals, adj_row, adj_col):
    from concourse.bass_utils import run_bass_kernel_spmd

    nc = _build_program()
    per_core, fr, fc, fv = _prep(
        np.asarray(adj_row), np.asarray(adj_col), np.asarray(adj_vals)
    )
    x = np.concatenate([np.asarray(user_emb), np.asarray(item_emb)], axis=0).astype(
        np.float32
    )
    for _ in range(NLAYERS):
        in_maps = [{"x": x, **per_core[c]} for c in range(NCORES)]
        res = run_bass_kernel_spmd(nc, in_maps, core_ids=list(range(NCORES))).results
        y = np.empty((N_NODES, DIM), np.float32)
        for c in range(NCORES):
            y[c * RPC:(c + 1) * RPC] = res[c]["y"][:RPC]
        if len(fr):
            np.add.at(y, fr, fv[:, None] * x[fc])
        x = y
    return x



# revision 3
# speedup vs baseline: 10.7651x; 10.7651x over previous
"""LightGCN 3-layer SpMM on 8 TRN2 NeuronCores — single SPMD launch.

Column-sharded edge-parallel SpMM: core c owns source columns
[c*12512, (c+1)*12512) and holds only that x-shard (3.2MB), so column
indices fit int16 natively. Each layer, every core SWDGE-gathers its
edges' source rows from its shard, scales on the vector engine, and
SWDGE-scatter-adds into a full-size partial output in DRAM; a
ReduceScatter across the 8 cores sums the partials and hands each core
its x-shard for the next layer. All 3 layers run in ONE launch, so edge
data is staged once and nothing round-trips through the host between
layers.

Scatter-add duplicate hazard (CCE read-modify-write is not atomic within
one instruction): edges are bucketed by occurrence rank within their
destination row, so any 1024-token scatter instruction touches each row
at most once. Pad tokens carry val=0 and rows distinct from the real
rows of their window (a val=0 lost update is harmless only against other
val=0 writes, so pads avoid real rows). Consecutive scatter instructions
are serialized by the tile framework's WAW tracking on the partial-y
tensor.
"""
import sys

sys.path.insert(0, "/opt/trn_rl_repo")
import numpy as np

N_NODES = 100000
DIM = 64
NCORES = 8
NLAYERS = 3
CH = 12512                      # column/row shard size (8*CH = 100096 >= N)
NPAD = NCORES * CH
SUB = 1024                      # tokens per gather/scatter instruction
BW = 16                         # windows per SBUF tile group

_prog_cache = {}


def _build_program(nw, chunk_map):
    key = (nw, tuple(chunk_map))
    if key in _prog_cache:
        return _prog_cache[key]
    from concourse import bass, bacc, tile, library_config, mybir

    f32 = mybir.dt.float32
    i16 = mybir.dt.int16
    nc = bacc.Bacc(None, target_bir_lowering=False, debug=False, num_devices=NCORES)
    xs0 = nc.dram_tensor("xs0", [CH, DIM], f32, kind="ExternalInput")
    cidx = nc.dram_tensor("cidx", [16, nw * 64], i16, kind="ExternalInput")
    ridx = nc.dram_tensor("ridx", [16, nw * 64], i16, kind="ExternalInput")
    vals = nc.dram_tensor("vals", [128, nw * 8], f32, kind="ExternalInput")
    yout = nc.dram_tensor("yout", [CH, DIM], f32, kind="ExternalOutput")
    groups = [list(range(NCORES))]

    with tile.TileContext(nc) as tc:
        nc.gpsimd.load_library(library_config.mlp)
        with (
            tc.tile_pool(name="dp", bufs=8, space="DRAM") as dp,
            tc.tile_pool(name="zp", bufs=1) as zp,
            tc.tile_pool(name="ip", bufs=3) as ip,
            tc.tile_pool(name="gp", bufs=3) as gp,
        ):
            ci_rep = dp.tile([128, nw * 64], i16)
            ri_rep = dp.tile([128, nw * 64], i16)
            ys = [
                dp.tile([NPAD, DIM], f32, name=f"ypart{i}") for i in range(NLAYERS)
            ]
            xs1 = dp.tile([CH, DIM], f32)
            xs2 = dp.tile([CH, DIM], f32)
            rs3 = dp.tile([CH, DIM], f32)

            # replicate the 16-partition-wrapped index images 8x for SWDGE
            for k in range(8):
                nc.sync.dma_start(ci_rep[16 * k:16 * (k + 1), :], cidx[:, :])
                nc.sync.dma_start(ri_rep[16 * k:16 * (k + 1), :], ridx[:, :])

            # zero the three partial-y buffers
            z = zp.tile([128, 2048], f32)
            nc.vector.memset(z[:], 0.0)
            rows_per = (128 * 2048) // DIM          # 4096 rows per 1MB store
            for y in ys:
                r0 = 0
                while r0 < NPAD:
                    n = min(rows_per, NPAD - r0)
                    nc.sync.dma_start(y[r0:r0 + n, :], z[:, :n * DIM // 128])
                    r0 += n

            srcs = [xs0, xs1, xs2]
            outs = [xs1, xs2, rs3]
            for L in range(NLAYERS):
                ysl = ys[L]
                for w0 in range(0, nw, BW):
                    ci = ip.tile([128, BW * 64], i16)
                    ri = ip.tile([128, BW * 64], i16)
                    vv = ip.tile([128, BW * 8, 1], f32)
                    nc.sync.dma_start(ci[:], ci_rep[:, w0 * 64:(w0 + BW) * 64])
                    nc.sync.dma_start(ri[:], ri_rep[:, w0 * 64:(w0 + BW) * 64])
                    nc.sync.dma_start(vv[:], vals[:, w0 * 8:(w0 + BW) * 8])
                    g = gp.tile([128, BW * 8, DIM], f32)
                    for wi in range(BW):
                        nc.gpsimd.dma_gather(
                            g[:, wi * 8:(wi + 1) * 8, :], srcs[L][:, :],
                            ci[:, wi * 64:(wi + 1) * 64], SUB, SUB, DIM,
                        )
                    ga, va = bass.broadcast_tensor_aps(g[:], vv[:])
                    nc.vector.tensor_tensor(ga, ga, va, mybir.AluOpType.mult)
                    for wi in range(BW):
                        s = chunk_map[w0 + wi]
                        nc.gpsimd.dma_scatter_add(
                            ysl[s * CH:(s + 1) * CH, :],
                            g[:, wi * 8:(wi + 1) * 8, :],
                            ri[:, wi * 64:(wi + 1) * 64], SUB, SUB, DIM,
                        )
                nc.gpsimd.collective_compute(
                    "ReduceScatter", mybir.AluOpType.add,
                    replica_groups=groups, ins=[ysl.opt()], outs=[outs[L].opt()],
                )
            nc.sync.dma_start(yout[:, :], rs3[:])
    nc.compile()
    _prog_cache[key] = nc
    return nc


def _prep_core(r, lc, v):
    """Edges of one core (global row r, local col lc, val v) ->
    (per-chunk window token lists, per-chunk window counts)."""
    s = r // CH
    lr = r - s * CH
    o = np.lexsort((lr, s))
    s, lr, lc, v = s[o], lr[o], lc[o], v[o]
    key = s * CH + lr
    new = np.r_[True, key[1:] != key[:-1]] if len(key) else np.zeros(0, bool)
    starts = np.flatnonzero(new)
    gid = np.cumsum(new) - 1
    k = np.arange(len(key)) - starts[gid]
    o2 = np.lexsort((lr, k, s))
    s, k, lr, lc, v = s[o2], k[o2], lr[o2], lc[o2], v[o2]

    chunks = []
    for ss in range(NCORES):
        msk = s == ss
        ks, lrs, lcs, vs = k[msk], lr[msk], lc[msk], v[msk]
        toks_r, toks_c, toks_v = [], [], []
        nwin = 0
        for b in np.unique(ks) if len(ks) else []:
            mb = ks == b
            rb, cb, vb = lrs[mb], lcs[mb], vs[mb]
            n = len(rb)
            npad = (-n) % SUB
            if npad:
                tail = rb[n - (n % SUB):] if n % SUB else np.zeros(0, np.int64)
                free = np.setdiff1d(
                    np.arange(npad + len(tail) + 1, dtype=np.int64), tail
                )[:npad]
                rb = np.r_[rb, free]
                cb = np.r_[cb, np.zeros(npad, np.int64)]
                vb = np.r_[vb, np.zeros(npad, np.float32)]
            toks_r.append(rb)
            toks_c.append(cb)
            toks_v.append(vb)
            nwin += len(rb) // SUB
        chunks.append((toks_r, toks_c, toks_v, nwin))
    return chunks


def _pad_window_tokens(n):
    """n all-pad windows: distinct rows per window, col 0, val 0."""
    rr = np.tile(np.arange(SUB, dtype=np.int64), n)
    return rr, np.zeros(n * SUB, np.int64), np.zeros(n * SUB, np.float32)


def _prep(adj_row, adj_col, adj_vals):
    r = np.asarray(adj_row).astype(np.int64)
    c = np.asarray(adj_col).astype(np.int64)
    v = np.asarray(adj_vals).astype(np.float32)
    core = c // CH
    per_core = []
    for cc in range(NCORES):
        m = core == cc
        per_core.append(_prep_core(r[m], c[m] - cc * CH, v[m]))

    nw_s = [
        max(per_core[cc][ss][3] for cc in range(NCORES)) for ss in range(NCORES)
    ]
    nw = sum(nw_s)
    nw_round = -(-nw // BW) * BW
    extra = nw_round - nw
    chunk_map = np.concatenate(
        [np.full(nw_s[ss], ss, np.int64) for ss in range(NCORES)]
        + [np.full(extra, NCORES - 1, np.int64)]
    )

    data = []
    for cc in range(NCORES):
        tr, tc_, tv = [], [], []
        for ss in range(NCORES):
            toks_r, toks_c, toks_v, nwin = per_core[cc][ss]
            tr += toks_r
            tc_ += toks_c
            tv += toks_v
            if nwin < nw_s[ss]:
                pr, pc, pv = _pad_window_tokens(nw_s[ss] - nwin)
                tr.append(pr)
                tc_.append(pc)
                tv.append(pv)
        if extra:
            pr, pc, pv = _pad_window_tokens(extra)
            tr.append(pr)
            tc_.append(pc)
            tv.append(pv)
        R = np.concatenate(tr)
        C = np.concatenate(tc_)
        V = np.concatenate(tv)
        assert len(R) == nw_round * SUB, (len(R), nw_round * SUB)
        cidx = np.ascontiguousarray(
            C.astype(np.int16).reshape(nw_round, SUB // 16, 16)
            .transpose(2, 0, 1).reshape(16, nw_round * 64)
        )
        ridx = np.ascontiguousarray(
            R.astype(np.int16).reshape(nw_round, SUB // 16, 16)
            .transpose(2, 0, 1).reshape(16, nw_round * 64)
        )
        vals = np.ascontiguousarray(
            V.reshape(nw_round, SUB // 128, 128)
            .transpose(2, 0, 1).reshape(128, nw_round * 8)
        )
        data.append({"cidx": cidx, "ridx": ridx, "vals": vals})
    return nw_round, chunk_map, data


def _run_cached(nc, in_maps):
    """run_bass_kernel_spmd's axon path, with the jitted callable cached."""
    import jax
    from jax.sharding import Mesh, PartitionSpec
    from jax.experimental.shard_map import shard_map
    from concourse import bass2jax, mybir

    cache = _prog_cache.setdefault("jit", {})
    if nc not in cache:
        bass2jax.install_neuronx_cc_hook()
        in_names, out_names, out_avals, zero_shapes = [], [], [], []
        partition_name = nc.partition_id_tensor.name if nc.partition_id_tensor else None
        for alloc in nc.m.functions[0].allocations:
            if not isinstance(alloc, mybir.MemoryLocationSet):
                continue
            name = alloc.memorylocations[0].name
            if alloc.kind == "ExternalInput":
                if name != partition_name:
                    in_names.append(name)
            elif alloc.kind == "ExternalOutput":
                out_names.append(name)
                shape = tuple(alloc.tensor_shape)
                dtype = mybir.dt.np(alloc.dtype)
                out_avals.append(jax.core.ShapedArray(shape, dtype))
                zero_shapes.append((shape, dtype))
        n_params = len(in_names)
        n_outs = len(out_avals)
        all_in = list(in_names) + list(out_names)
        if partition_name is not None:
            all_in.append(partition_name)

        def _body(*args):
            operands = list(args)
            if partition_name is not None:
                operands.append(bass2jax.partition_id_tensor())
            outs = bass2jax._bass_exec_p.bind(
                *operands,
                out_avals=tuple(out_avals),
                in_names=tuple(all_in),
                out_names=tuple(out_names),
                lowering_input_output_aliases=(),
                sim_require_finite=True,
                sim_require_nnan=True,
                nc=nc,
            )
            return tuple(outs)

        devices = jax.devices()[:NCORES]
        mesh = Mesh(np.asarray(devices), ("core",))
        fn = jax.jit(
            shard_map(
                _body, mesh=mesh,
                in_specs=(PartitionSpec("core"),) * (n_params + n_outs),
                out_specs=(PartitionSpec("core"),) * n_outs,
                check_rep=False,
            ),
            donate_argnums=tuple(range(n_params, n_params + n_outs)),
            keep_unused=True,
        )
        cache[nc] = (fn, in_names[:n_params], out_names, out_avals, zero_shapes)
    fn, in_names, out_names, out_avals, zero_shapes = cache[nc]
    concat_in = [
        np.concatenate([np.asarray(m[name]) for m in in_maps], axis=0)
        for name in in_names
    ]
    concat_zeros = [
        np.zeros((NCORES * s[0], *s[1:]), d) for (s, d) in zero_shapes
    ]
    out_arrs = fn(*concat_in, *concat_zeros)
    return [
        {
            name: np.asarray(out_arrs[i]).reshape(NCORES, *out_avals[i].shape)[cc]
            for i, name in enumerate(out_names)
        }
        for cc in range(NCORES)
    ]


def kernel(user_emb, item_emb, adj_vals, adj_row, adj_col):
    nw, chunk_map, data = _prep(adj_row, adj_col, adj_vals)
    nc = _build_program(nw, chunk_map)

    x0 = np.zeros((NPAD, DIM), np.float32)
    x0[:N_NODES // 2] = np.asarray(user_emb)
    x0[N_NODES // 2:N_NODES] = np.asarray(item_emb)

    in_maps = [
        {"xs0": x0[cc * CH:(cc + 1) * CH], **data[cc]} for cc in range(NCORES)
    ]
    res = _run_cached(nc, in_maps)
    y = np.concatenate([res[cc]["yout"] for cc in range(NCORES)], axis=0)
    return np.ascontiguousarray(y[:N_NODES])


# revision 4
# speedup vs baseline: 20.8642x; 1.9381x over previous
"""LightGCN 3-layer SpMM on 8 TRN2 NeuronCores — single SPMD launch.

Column-sharded edge-parallel SpMM: core c owns source columns
[c*12512, (c+1)*12512) and holds only that x-shard, so column indices
fit int16 natively. Each layer, every core SWDGE-gathers its edges'
source rows from its shard, scales on the vector engine, and
SWDGE-scatter-adds into a full-size partial output in DRAM; a
ReduceScatter across the 8 cores sums the partials and hands each core
its x-shard for the next layer. All 3 layers run in ONE launch, so edge
data is staged once and nothing round-trips through the host between
layers. x/vals/output are staged bf16 and cast on-device (SWDGE DMA
cast); donated output buffers are created device-side.

Scatter-add duplicate hazard (CCE read-modify-write is not atomic within
one instruction): edges are packed into 1024-token windows such that a
window touches each destination row at most once (round-robin by
occurrence rank with overflow redistribution). Pad tokens carry val=0
and rows distinct from the real rows of their window. Same-destination
windows are serialized by the tile framework's WAW tracking on the
partial-y tensor; window order interleaves the 8 destination row-chunks
so range-aware tracking can overlap independent scatters.
"""
import sys

sys.path.insert(0, "/opt/trn_rl_repo")
import numpy as np

N_NODES = 100000
DIM = 64
NCORES = 8
NLAYERS = 3
CH = 12512                      # column/row shard size (8*CH = 100096 >= N)
NPAD = NCORES * CH
SUB = 1024                      # tokens per gather/scatter instruction
BW = 16                        # windows per SBUF tile group (2 chunk rounds)

_prog_cache = {}


def _build_program(nw):
    key = ("prog", nw)
    if key in _prog_cache:
        return _prog_cache[key]
    from concourse import bass, bacc, tile, library_config, mybir

    f32 = mybir.dt.float32
    bf16 = mybir.dt.bfloat16
    i16 = mybir.dt.int16
    nc = bacc.Bacc(
        None, target_bir_lowering=False, debug=False, num_devices=NCORES,
        num_swdge_queues=2,
    )
    xs0 = nc.dram_tensor("xs0", [CH, DIM], bf16, kind="ExternalInput")
    cidx = nc.dram_tensor("cidx", [16, nw * 64], i16, kind="ExternalInput")
    ridx = nc.dram_tensor("ridx", [16, nw * 64], i16, kind="ExternalInput")
    vals = nc.dram_tensor("vals", [128, nw * 8], bf16, kind="ExternalInput")
    yout = nc.dram_tensor("yout", [CH, DIM], bf16, kind="ExternalOutput")
    groups = [list(range(NCORES))]

    with tile.TileContext(nc) as tc:
        nc.gpsimd.load_library(library_config.mlp)
        with (
            tc.tile_pool(name="dp", bufs=10, space="DRAM") as dp,
            tc.tile_pool(name="zp", bufs=1) as zp,
            tc.tile_pool(name="ip", bufs=3) as ip,
            tc.tile_pool(name="gp", bufs=3) as gp,
        ):
            ci_rep = dp.tile([128, nw * 64], i16)
            ri_rep = dp.tile([128, nw * 64], i16)
            ys = [
                dp.tile([NPAD, DIM], f32, name=f"ypart{i}") for i in range(NLAYERS)
            ]
            xsf = dp.tile([CH, DIM], f32)
            valsf = dp.tile([128, nw * 8], f32)
            xs1 = dp.tile([CH, DIM], f32)
            xs2 = dp.tile([CH, DIM], f32)
            rs3 = dp.tile([CH, DIM], f32)

            # bf16 -> f32 casts during DRAM->DRAM SWDGE DMA
            nc.gpsimd.dma_start(xsf[:, :], xs0[:, :])
            nc.gpsimd.dma_start(valsf[:], vals[:, :])

            # replicate the 16-partition-wrapped index images 8x for SWDGE
            for k in range(8):
                nc.sync.dma_start(ci_rep[16 * k:16 * (k + 1), :], cidx[:, :])
                nc.sync.dma_start(ri_rep[16 * k:16 * (k + 1), :], ridx[:, :])

            # zero the three partial-y buffers
            z = zp.tile([128, 2048], f32)
            nc.vector.memset(z[:], 0.0)
            rows_per = (128 * 2048) // DIM          # 4096 rows per 1MB store
            for y in ys:
                r0 = 0
                while r0 < NPAD:
                    n = min(rows_per, NPAD - r0)
                    nc.sync.dma_start(y[r0:r0 + n, :], z[:, :n * DIM // 128])
                    r0 += n

            srcs = [xsf, xs1, xs2]
            outs = [xs1, xs2, rs3]
            for L in range(NLAYERS):
                ysl = ys[L]
                for w0 in range(0, nw, BW):
                    ci = ip.tile([128, BW * 64], i16)
                    ri = ip.tile([128, BW * 64], i16)
                    vv = ip.tile([128, BW * 8, 1], f32)
                    nc.sync.dma_start(ci[:], ci_rep[:, w0 * 64:(w0 + BW) * 64])
                    nc.sync.dma_start(ri[:], ri_rep[:, w0 * 64:(w0 + BW) * 64])
                    nc.sync.dma_start(vv[:], valsf[:, w0 * 8:(w0 + BW) * 8])
                    g = gp.tile([128, BW * 8, DIM], f32)
                    for wi in range(BW):
                        nc.gpsimd.dma_gather(
                            g[:, wi * 8:(wi + 1) * 8, :], srcs[L][:, :],
                            ci[:, wi * 64:(wi + 1) * 64], SUB, SUB, DIM,
                            queue_num=0,
                        )
                    ga, va = bass.broadcast_tensor_aps(g[:], vv[:])
                    nc.vector.tensor_tensor(ga, ga, va, mybir.AluOpType.mult)
                    for wi in range(BW):
                        s = (w0 + wi) % NCORES
                        nc.gpsimd.dma_scatter_add(
                            ysl[s * CH:(s + 1) * CH, :],
                            g[:, wi * 8:(wi + 1) * 8, :],
                            ri[:, wi * 64:(wi + 1) * 64], SUB, SUB, DIM,
                            queue_num=1,
                        )
                nc.gpsimd.collective_compute(
                    "ReduceScatter", mybir.AluOpType.add,
                    replica_groups=groups, ins=[ysl.opt()], outs=[outs[L].opt()],
                )
            # f32 -> bf16 cast during DMA
            nc.gpsimd.dma_start(yout[:, :], rs3[:, :])
    nc.compile()
    _prog_cache[key] = nc
    return nc


def _pack_chunk(lr, lc, v, W):
    """Pack one (core, row-chunk)'s edges into W windows of SUB tokens with
    distinct rows per window. Returns (R, C, V) flat arrays of W*SUB."""
    E = len(lr)
    assert E <= W * SUB
    if E:
        o = np.argsort(lr, kind="stable")
        lr, lc, v = lr[o], lc[o], v[o]
        new = np.r_[True, lr[1:] != lr[:-1]]
        starts = np.flatnonzero(new)
        gid = np.cumsum(new) - 1
        k = np.arange(E) - starts[gid]
        assert k.max() < W, "row degree exceeds window count"
        assign = (k + lr) % W
        counts = np.bincount(assign, minlength=W)
        present = np.zeros((W, CH), bool)
        present[assign, lr] = True
        if (counts > SUB).any():
            by_w = np.argsort(assign, kind="stable")
            bounds = np.searchsorted(assign[by_w], np.arange(W + 1))
            for w in np.flatnonzero(counts > SUB):
                excess = by_w[bounds[w]:bounds[w + 1]][SUB - counts[w]:]
                for e in excess:
                    cand = np.flatnonzero((counts < SUB) & ~present[:, lr[e]])
                    assert len(cand), "window packing infeasible"
                    t = cand[0]
                    present[assign[e], lr[e]] = False
                    counts[assign[e]] -= 1
                    assign[e] = t
                    present[t, lr[e]] = True
                    counts[t] += 1
    else:
        assign = np.zeros(0, np.int64)
        counts = np.zeros(W, np.int64)

    R = np.zeros((W, SUB), np.int64)
    C = np.zeros((W, SUB), np.int64)
    V = np.zeros((W, SUB), np.float32)
    by_w = np.argsort(assign, kind="stable")
    bounds = np.searchsorted(assign[by_w], np.arange(W + 1))
    for w in range(W):
        es = by_w[bounds[w]:bounds[w + 1]]
        n = len(es)
        R[w, :n] = lr[es]
        C[w, :n] = lc[es]
        V[w, :n] = v[es]
        need = SUB - n
        if need:
            free = np.setdiff1d(
                np.arange(need + n + 1, dtype=np.int64), lr[es]
            )[:need]
            R[w, n:] = free
    return R.reshape(-1), C.reshape(-1), V.reshape(-1)


def _prep(adj_row, adj_col, adj_vals):
    """Returns (nw, per-core input dict list). Window j targets row-chunk
    j % 8 (chunk-interleaved order)."""
    r = np.asarray(adj_row).astype(np.int64)
    c = np.asarray(adj_col).astype(np.int64)
    v = np.asarray(adj_vals).astype(np.float32)
    core = c // CH
    edges = []
    nw_u = 0
    for cc in range(NCORES):
        m = core == cc
        rc, lcc, vc = r[m], c[m] - cc * CH, v[m]
        s = rc // CH
        per_chunk = []
        for ss in range(NCORES):
            ms = s == ss
            lr = rc[ms] - ss * CH
            per_chunk.append((lr, lcc[ms], vc[ms]))
            wmin = -(-len(lr) // SUB)
            if len(lr):
                deg = np.bincount(lr).max()
                wmin = max(wmin, deg)
            nw_u = max(nw_u, wmin)
        edges.append(per_chunk)
    nw_u += nw_u % 2                        # NW = 8*nw_u divisible by BW=16
    nw = NCORES * nw_u

    import ml_dtypes

    data = []
    for cc in range(NCORES):
        Rs, Cs, Vs = [], [], []
        for ss in range(NCORES):
            lr, lc_, vv = edges[cc][ss]
            R, C, V = _pack_chunk(lr, lc_, vv, nw_u)
            Rs.append(R)
            Cs.append(C)
            Vs.append(V)
        # interleave chunks: window j -> chunk j % 8, local window j // 8
        R = np.stack(Rs).reshape(NCORES, nw_u, SUB).transpose(1, 0, 2).reshape(-1)
        C = np.stack(Cs).reshape(NCORES, nw_u, SUB).transpose(1, 0, 2).reshape(-1)
        V = np.stack(Vs).reshape(NCORES, nw_u, SUB).transpose(1, 0, 2).reshape(-1)
        cidx = np.ascontiguousarray(
            C.astype(np.int16).reshape(nw, SUB // 16, 16)
            .transpose(2, 0, 1).reshape(16, nw * 64)
        )
        ridx = np.ascontiguousarray(
            R.astype(np.int16).reshape(nw, SUB // 16, 16)
            .transpose(2, 0, 1).reshape(16, nw * 64)
        )
        vals = np.ascontiguousarray(
            V.reshape(nw, SUB // 128, 128)
            .transpose(2, 0, 1).reshape(128, nw * 8)
        ).astype(ml_dtypes.bfloat16)
        data.append({"cidx": cidx, "ridx": ridx, "vals": vals})
    return nw, data


def _make_in_maps(user_emb, item_emb, data):
    import ml_dtypes

    x0 = np.zeros((NPAD, DIM), np.float32)
    x0[:N_NODES // 2] = np.asarray(user_emb)
    x0[N_NODES // 2:N_NODES] = np.asarray(item_emb)
    x0 = x0.astype(ml_dtypes.bfloat16)
    return [
        {"xs0": x0[cc * CH:(cc + 1) * CH], **data[cc]} for cc in range(NCORES)
    ]


def _run_cached(nc, in_maps):
    """run_bass_kernel_spmd's axon path, with the jitted callable cached and
    donated output buffers created device-side."""
    import jax
    import jax.numpy as jnp
    from jax.sharding import Mesh, PartitionSpec, NamedSharding
    from jax.experimental.shard_map import shard_map
    from concourse import bass2jax, mybir

    cache = _prog_cache.setdefault("jit", {})
    if nc not in cache:
        bass2jax.install_neuronx_cc_hook()
        in_names, out_names, out_avals, zero_shapes = [], [], [], []
        partition_name = nc.partition_id_tensor.name if nc.partition_id_tensor else None
        for alloc in nc.m.functions[0].allocations:
            if not isinstance(alloc, mybir.MemoryLocationSet):
                continue
            name = alloc.memorylocations[0].name
            if alloc.kind == "ExternalInput":
                if name != partition_name:
                    in_names.append(name)
            elif alloc.kind == "ExternalOutput":
                out_names.append(name)
                shape = tuple(alloc.tensor_shape)
                dtype = mybir.dt.np(alloc.dtype)
                out_avals.append(jax.core.ShapedArray(shape, dtype))
                zero_shapes.append((shape, dtype))
        n_params = len(in_names)
        n_outs = len(out_avals)
        all_in = list(in_names) + list(out_names)
        if partition_name is not None:
            all_in.append(partition_name)

        def _body(*args):
            operands = list(args)
            if partition_name is not None:
                operands.append(bass2jax.partition_id_tensor())
            outs = bass2jax._bass_exec_p.bind(
                *operands,
                out_avals=tuple(out_avals),
                in_names=tuple(all_in),
                out_names=tuple(out_names),
                lowering_input_output_aliases=(),
                sim_require_finite=True,
                sim_require_nnan=True,
                nc=nc,
            )
            return tuple(outs)

        devices = jax.devices()[:NCORES]
        mesh = Mesh(np.asarray(devices), ("core",))
        fn = jax.jit(
            shard_map(
                _body, mesh=mesh,
                in_specs=(PartitionSpec("core"),) * (n_params + n_outs),
                out_specs=(PartitionSpec("core"),) * n_outs,
                check_rep=False,
            ),
            donate_argnums=tuple(range(n_params, n_params + n_outs)),
            keep_unused=True,
        )
        shardings = tuple(
            NamedSharding(mesh, PartitionSpec("core")) for _ in zero_shapes
        )
        zfn = jax.jit(
            lambda: tuple(
                jnp.zeros((NCORES * s[0], *s[1:]), d) for (s, d) in zero_shapes
            ),
            out_shardings=shardings,
        )
        cache[nc] = (fn, zfn, in_names[:n_params], out_names, out_avals)
    fn, zfn, in_names, out_names, out_avals = cache[nc]
    concat_in = [
        np.concatenate([np.asarray(m[name]) for m in in_maps], axis=0)
        for name in in_names
    ]
    out_arrs = fn(*concat_in, *zfn())
    return [
        {
            name: np.asarray(out_arrs[i]).reshape(NCORES, *out_avals[i].shape)[cc]
            for i, name in enumerate(out_names)
        }
        for cc in range(NCORES)
    ]


def kernel(user_emb, item_emb, adj_vals, adj_row, adj_col):
    nw, data = _prep(adj_row, adj_col, adj_vals)
    nc = _build_program(nw)
    in_maps = _make_in_maps(user_emb, item_emb, data)
    res = _run_cached(nc, in_maps)
    y = np.concatenate(
        [res[cc]["yout"].astype(np.float32) for cc in range(NCORES)], axis=0
    )
    return np.ascontiguousarray(y[:N_NODES])
